# revision 1
# baseline (speedup 1.0000x reference)
"""Trainium2 Bass kernel: additive (Bahdanau-style) attention readout.

Reference computation (per batch b):
    energy  = tanh(enc @ W1.T + dec_b @ W2.T + W_b)      # (S, H)
    scores  = energy @ V + V_b, masked                   # (S,)
    attn    = softmax(scores)                            # (S,)
    context = attn @ enc                                 # (D,)

Sharding: data-parallel over batch across 8 NeuronCores (4 batches/core),
small weights replicated.  Host prep: enc cast to fp16 (pre-transposed to
[d, s] so et tiles load contiguously), W1.T pre-transposed (fp16), the tiny
dec projection + bias folded to a per-(h,batch) bias, and the mask + V_b
folded to an additive score penalty.

Device dataflow per batch (cost-model span ~552 us/core, PE-bound):
  - et tiles [p=d, k, s] stream in per k-chunk on the SP HWDGE queue.
  - pass1: psum[h,s] = sum_k W1T-chunk.T @ et-chunk   (fp16 PE, N=512)
  - tanh+bias on ScalarE (psum -> sbuf fp16), scores matmul V.T @ energy
    on PE (fp16), + penalty row on DVE.
  - softmax on the [1, S] score row (DVE reduce + ScalarE exp w/ accum),
    then attn bounced to DRAM (ACT HWDGE queue, to keep SP streaming).
  - context pass2: batches 0..bpc-2 on DVE (mult) + ScalarE (accum-reduce)
    over the resident transposed tiles — fully hidden under the next
    batch's pass1; the LAST batch runs on the then-idle PE against a
    host-shipped natural-layout slice, shortening the kernel tail.
"""

import numpy as np
import ml_dtypes

import concourse.bass as bass
import concourse.tile as tile
from concourse import bacc, mybir
from concourse.bass_utils import run_bass_kernel_spmd

# Problem shapes (hardcoded per contract).
B, S, D, H = 32, 2048, 2048, 1024
NCORES = 8
BPC = B // NCORES  # batches per core

F32 = mybir.dt.float32
F32R = mybir.dt.float32r
BF16 = mybir.dt.bfloat16
F16 = mybir.dt.float16
AF = mybir.ActivationFunctionType
ALU = mybir.AluOpType


def build_program(bpc=BPC, s=S, d=D, h=H, nt=512, nhalf=2, host_t=False):
    """Build the per-core Bass program (SPMD; identical on all cores).

    host_t: if True, enc arrives pre-transposed from the host as
    [bpc, d, s] and et tiles load with plain DMAs (no xbar transpose).
    """
    P = 128
    KD = d // P            # enc-feature chunks (contraction of pass1)
    MH = h // P            # h chunks
    sh = s // nhalf        # tokens per s-half (et tile granularity)
    assert sh % nt == 0 and d % P == 0 and h % P == 0
    NTH = sh // nt         # token tiles per half

    nc = bacc.Bacc(None, target_bir_lowering=False)
    enc_shape = [bpc, d, s] if host_t else [bpc, s, d]
    enc = nc.declare_dram_parameter("enc", enc_shape, F16, isOutput=False)
    w1t = nc.declare_dram_parameter("w1t", [d, h], F16, isOutput=False)
    vt = nc.declare_dram_parameter("vt", [h], F16, isOutput=False)
    cbias = nc.declare_dram_parameter("cbias", [h, bpc], F32, isOutput=False)
    pen = nc.declare_dram_parameter("pen", [bpc, s], F32, isOutput=False)
    # natural-layout copy of the core's LAST batch, for the PE-based pass2
    # that shortens the kernel tail
    encn = nc.declare_dram_parameter("encn", [s, d], F16, isOutput=False)
    ctx_out = nc.declare_dram_parameter("ctx", [bpc, d], F32, isOutput=True)
    attn_dram = nc.dram_tensor("attn_bounce", [s], F32)

    with tile.TileContext(nc) as tc:
        with (
            tc.tile_pool(name="singles", bufs=1) as singles,
            tc.tile_pool(name="et_pool", bufs=3) as et_pool,
            tc.tile_pool(name="en_pool", bufs=3) as en_pool,
            tc.tile_pool(name="row_pool", bufs=2) as row_pool,
            tc.tile_pool(name="pen_pool", bufs=2) as pen_pool,
            tc.tile_pool(name="bc_pool", bufs=1) as bc_pool,
            tc.tile_pool(name="scr_pool", bufs=2) as scr_pool,
            tc.tile_pool(name="ctx_pool", bufs=2) as ctx_pool,
            tc.tile_pool(name="stat_pool", bufs=4) as stat_pool,
            tc.tile_pool(name="psum_mm", bufs=2, space="PSUM") as psum_mm,
            tc.tile_pool(name="psum_sc", bufs=2, space="PSUM") as psum_sc,
            tc.tile_pool(name="psum_ctx", bufs=1, space="PSUM") as psum_ctx,
        ):
            # Resident constants.  w1 is loaded per k-chunk on the gpsimd
            # queue so the SP queue can start streaming et immediately and
            # the first matmuls only wait for their own chunks.
            w1_sb = singles.tile([P, KD, h], F16)
            w1_r = w1t.rearrange("(ko p) h -> p ko h", p=P)
            w1ch = min(4, KD)
            for k in range(0, KD, w1ch):
                nc.gpsimd.dma_start(
                    w1_sb[:, k:k + w1ch, :], w1_r[:, k:k + w1ch, :]
                )
            vt_sb = singles.tile([P, MH], F16)
            nc.gpsimd.dma_start(vt_sb, vt.rearrange("(m p) -> p m", p=P))
            cb_sb = singles.tile([P, MH, bpc], F32)
            nc.gpsimd.dma_start(cb_sb, cbias.rearrange("(m p) b -> p m b", p=P))

            for b in range(bpc):
                pen_row = pen_pool.tile([1, s], F32, tag="pen")
                nc.sync.dma_start(pen_row, pen[b][None, :])

                row = row_pool.tile([1, s], F32, tag="row")
                ets = []
                for hf in range(nhalf):
                    # Transposed enc tiles for this s-half:
                    # et[p, k, t] = enc[b, hf*sh + t, k*P + p]
                    # Loaded in k-chunks so pass1's k-loop can start before
                    # the whole half has landed.
                    et = et_pool.tile([P, KD, sh], F16, tag="et")
                    KCH = min(4, KD)
                    for kc in range(0, KD, KCH):
                        if host_t:
                            nc.sync.dma_start(
                                et[:, kc:kc + KCH, :],
                                enc[
                                    b, kc * P:(kc + KCH) * P,
                                    hf * sh:(hf + 1) * sh,
                                ].rearrange("(ko p) t -> p ko t", p=P),
                            )
                        else:
                            for k in range(kc, kc + KCH):
                                nc.sync.dma_start_transpose(
                                    et[:, k, :],
                                    enc[
                                        b, hf * sh:(hf + 1) * sh,
                                        k * P:(k + 1) * P,
                                    ],
                                )
                    ets.append(et)
                    for n in range(NTH):
                        ng = hf * NTH + n  # global token-tile index
                        ps_sc = psum_sc.tile([1, nt], F32)
                        for m in range(MH):
                            ps = psum_mm.tile([P, nt], F32)
                            for k in range(KD):
                                nc.tensor.matmul(
                                    ps,
                                    w1_sb[:, k, m * P:(m + 1) * P],
                                    et[:, k, n * nt:(n + 1) * nt],
                                    start=(k == 0),
                                    stop=(k == KD - 1),
                                )
                            energy = en_pool.tile([P, nt], F16, tag="energy")
                            nc.scalar.activation(
                                energy, ps, AF.Tanh,
                                bias=cb_sb[:, m, b:b + 1], scale=1.0,
                            )
                            nc.tensor.matmul(
                                ps_sc,
                                vt_sb[:, m:m + 1],
                                energy,
                                start=(m == 0),
                                stop=(m == MH - 1),
                            )
                        # scores(+V_b, +mask penalty) into the batch row
                        nc.vector.tensor_tensor(
                            row[:, ng * nt:(ng + 1) * nt],
                            ps_sc,
                            pen_row[:, ng * nt:(ng + 1) * nt],
                            ALU.add,
                        )

                # Softmax over the full row (in place: row -> exp -> attn).
                negmax = stat_pool.tile([1, 1], F32, tag="negmax")
                nc.vector.tensor_reduce(
                    negmax, row, axis=mybir.AxisListType.X, op=ALU.max,
                    negate=True,
                )
                ssum = stat_pool.tile([1, 1], F32, tag="ssum")
                nc.scalar.activation(
                    row, row, AF.Exp, bias=negmax, scale=1.0, accum_out=ssum,
                )
                rinv = stat_pool.tile([1, 1], F32, tag="rinv")
                nc.vector.reciprocal(rinv, ssum)
                nc.vector.tensor_scalar_mul(row, row, rinv)

                # attn bounce to DRAM, on the ACT HWDGE queue so the SP
                # queue stays a pure stream of et loads.
                nc.scalar.dma_start(attn_dram[None, :], row)

                if b == bpc - 1:
                    # Last batch: pass2 on the (otherwise idle) PE using the
                    # natural-layout copy.  attn read back partition-major,
                    # cast fp32 -> fp16 during the SWDGE DMA.
                    SK = s // P
                    attn_part = stat_pool.tile([P, SK], F16, tag="attn_part")
                    nc.gpsimd.dma_start(
                        attn_part,
                        attn_dram[:].rearrange("(sk p) -> p sk", p=P),
                    )
                    ctx_ps = psum_ctx.tile([1, d], F32)
                    # at nt=512 each slice is exactly one 2KB zero region;
                    # only smaller (test) shapes need the check skipped
                    skipg = nt * 4 < 2048
                    NJ = min(4, SK)
                    skg = SK // NJ
                    for j in range(NJ):
                        ent = et_pool.tile([P, skg, d], F16, tag="et")
                        nc.sync.dma_start(
                            ent,
                            encn[j * skg * P:(j + 1) * skg * P, :].rearrange(
                                "(c p) dd -> p c dd", p=P
                            ),
                        )
                        for c in range(skg):
                            sk = j * skg + c
                            for dt_ in range(d // nt):
                                nc.tensor.matmul(
                                    ctx_ps[:, dt_ * nt:(dt_ + 1) * nt],
                                    attn_part[:, sk:sk + 1],
                                    ent[:, c, dt_ * nt:(dt_ + 1) * nt],
                                    start=(sk == 0),
                                    stop=(sk == SK - 1),
                                    skip_group_check=skipg,
                                )
                    ctx_row = ctx_pool.tile([1, d], F32, tag="ctxrow")
                    nc.vector.tensor_copy(ctx_row, ctx_ps)
                    nc.scalar.dma_start(ctx_out[b][None, :], ctx_row)
                else:
                    # Broadcast attn across partitions via a replicated
                    # (partition-step-0) SWDGE read.
                    attn_bc = bc_pool.tile([P, s], F32, tag="attn_bc")
                    attn_src = attn_dram[None, :]
                    attn_src = bass.AP(
                        tensor=attn_src.tensor,
                        offset=attn_src.offset,
                        ap=[[0, P]] + list(attn_src.ap[1:]),
                    )
                    nc.gpsimd.dma_start(attn_bc, attn_src)

                    # Pass 2: context[d] = sum_s attn[s] * enc[s, d] on DVE,
                    # reusing the resident transposed tiles.
                    # (TensorTensorReduce is not supported by this runtime,
                    # so multiply + reduce.)  hf outer so each half tile is
                    # fully consumed (and its pool slot released for batch
                    # b+1) as early as possible.
                    ctx_sb = ctx_pool.tile([P, KD], F32, tag="ctx")
                    for hf in range(nhalf):
                        for k in range(KD):
                            scratch = scr_pool.tile(
                                [P, sh], F32, tag="scratch"
                            )
                            nc.vector.tensor_tensor(
                                scratch,
                                ets[hf][:, k, :],
                                attn_bc[:, hf * sh:(hf + 1) * sh],
                                ALU.mult,
                            )
                            part = stat_pool.tile([P, 1], F32, tag="part")
                            # reduce on ScalarE (in-place copy + accumulator)
                            # so DVE only does the multiplies.
                            nc.scalar.activation(
                                scratch, scratch, AF.Copy, scale=1.0,
                                accum_out=part,
                            )
                            if hf == 0:
                                nc.vector.tensor_copy(
                                    ctx_sb[:, k:k + 1], part
                                )
                            else:
                                nc.vector.tensor_tensor(
                                    ctx_sb[:, k:k + 1], ctx_sb[:, k:k + 1],
                                    part, ALU.add,
                                )
                    nc.scalar.dma_start(
                        ctx_out[b].rearrange("(k p) -> p k", p=P), ctx_sb,
                    )
    nc.finalize()
    return nc


_PROGRAM_CACHE = {}
HOST_TRANSPOSE = True


def _get_program(key, **kwargs):
    if key not in _PROGRAM_CACHE:
        _PROGRAM_CACHE[key] = build_program(**kwargs)
    return _PROGRAM_CACHE[key]


def prep_inputs(enc_output, enc_mask, dec_hidden, W_w, W_b, V_w, V_b):
    """Host-side shard + prep: returns per-core in_maps."""
    enc = np.asarray(enc_output, dtype=np.float32)
    mask = np.asarray(enc_mask, dtype=np.float32)[..., 0]          # (B, S)
    dec = np.asarray(dec_hidden, dtype=np.float32)[0]              # (B, H)
    W = np.asarray(W_w, dtype=np.float32)                          # (H, 3H)
    Wb = np.asarray(W_b, dtype=np.float32)                         # (H,)
    V = np.asarray(V_w, dtype=np.float32)[0]                       # (H,)
    Vb = float(np.asarray(V_b, dtype=np.float32)[0])

    enc_nat = enc.astype(np.float16)  # (B, S, D)
    if HOST_TRANSPOSE:
        enc_bf = np.ascontiguousarray(enc_nat.transpose(0, 2, 1))  # (B, D, S)
    else:
        enc_bf = enc_nat
    w1t = np.ascontiguousarray(W[:, :D].T).astype(np.float16)  # (D, H)
    # Tiny dec projection folded into a per-(h, b) bias (0.01% of FLOPs).
    cbias_all = (dec @ W[:, D:].T + Wb).astype(np.float32)         # (B, H)
    pen_all = (np.where(mask > 0, 0.0, -1e30) + Vb).astype(np.float32)  # (B, S)

    in_maps = []
    for c in range(NCORES):
        sl = slice(c * BPC, (c + 1) * BPC)
        in_maps.append({
            "enc": enc_bf[sl],
            "w1t": w1t,
            "vt": V.astype(np.float16),
            "cbias": np.ascontiguousarray(cbias_all[sl].T),        # (H, BPC)
            "pen": np.ascontiguousarray(pen_all[sl]),
            "encn": np.ascontiguousarray(enc_nat[c * BPC + BPC - 1]),
        })
    return in_maps


def kernel(**inputs) -> np.ndarray:
    in_maps = prep_inputs(**inputs)
    nc = _get_program(("full", HOST_TRANSPOSE), host_t=HOST_TRANSPOSE)
    res = run_bass_kernel_spmd(nc, in_maps, list(range(NCORES)))
    out = np.concatenate(
        [res.results[c]["ctx"] for c in range(NCORES)], axis=0
    )
    return np.ascontiguousarray(out.astype(np.float32))


if __name__ == "__main__":
    rng = np.random.default_rng(0)
    inputs = {
        "enc_output": rng.standard_normal((B, S, D), dtype=np.float32),
        "enc_mask": np.ones((B, S, 1), dtype=np.float32),
        "dec_hidden": rng.standard_normal((1, B, H), dtype=np.float32),
        "W_w": (rng.standard_normal((H, 3 * H), dtype=np.float32)
                / np.sqrt(3 * H)),
        "W_b": np.zeros((H,), dtype=np.float32),
        "V_w": rng.standard_normal((1, H), dtype=np.float32) / np.sqrt(H),
        "V_b": np.zeros((1,), dtype=np.float32),
    }
    out = kernel(**inputs)
    print(out.shape, out.dtype, float(np.abs(out).mean()))



# revision 3
# speedup vs baseline: 1.4611x; 1.4611x over previous
"""Trainium2 Bass kernel: additive (Bahdanau-style) attention readout.

Reference computation (per batch b):
    energy  = tanh(enc @ W1.T + dec_b @ W2.T + W_b)      # (S, H)
    scores  = energy @ V + V_b, masked                   # (S,)
    attn    = softmax(scores)                            # (S,)
    context = attn @ enc                                 # (D,)

Sharding: data-parallel over batch across 8 NeuronCores (4 batches/core),
small weights replicated.

Device dataflow (fp8 DoubleRow pass1, cost-model span ~275 us/core):
  - pass1 runs on the PE in fp8e4 DoubleRow mode (256-deep contraction,
    0.5 cyc/output column = 4x fp16 throughput).  enc is quantized to
    e4m3 on the host; W1 is pre-scaled by 64 and split into
    W8a = e4m3(64 W1) plus the residual W8b = e4m3(64 W1 - W8a), and both
    terms accumulate into the same PSUM group.  The residual removes the
    systematic W-quantization error (device rel-err ~1.2e-2 vs the 2e-2
    gate; enc quantization is the remaining error source).  tanh applies
    scale=1/64 to undo the W pre-scale, with the dec projection + bias
    folded per (h,b) into the activation bias.
  - scores stay fp16 (fp8 energy would add ~2.4e-2 error): V.T @ energy
    per m-chunk on the PE, software-pipelined one m-chunk behind pass1 so
    the in-order PE queue never stalls waiting for ACT's tanh.
  - softmax on the [1, S] row (DVE max / ACT exp+accum / DVE normalize),
    attn bounced to DRAM on the DVE HWDGE queue.
  - pass2 (context) needs >=fp16 enc (fp8 would put its 3.6% element
    noise straight on the output), so a separate fp16 transposed stream
    feeds fused multiply+accumulate scalar_tensor_tensor ops on the DVE,
    hidden under the next batch's pass1.  The LAST batch's pass2 is split
    between the then-idle PE (tokens [0, SPE*128) from a host-shipped
    natural-layout fp16 slice) and the DVE (remaining tokens); the two
    partial context vectors are summed on the host (free), which shortens
    the kernel tail.
  - queue map keeps every FIFO stall-free: SP = enc fp8 + natural slice,
    ACT = enc fp16 stream, DVE = attn bounce write, Pool/SWDGE = weights,
    penalty rows, attn broadcast reads (with f32->f16 cast) + ctx writes.
"""

import numpy as np
import ml_dtypes

import concourse.bass as bass
import concourse.tile as tile
from concourse import bacc, mybir
from concourse.bass_utils import run_bass_kernel_spmd

# Problem shapes (hardcoded per contract).
B, S, D, H = 32, 2048, 2048, 1024
NCORES = 8
BPC = B // NCORES  # batches per core

F32 = mybir.dt.float32
BF16 = mybir.dt.bfloat16
F16 = mybir.dt.float16
FP8 = mybir.dt.float8e4
AF = mybir.ActivationFunctionType
ALU = mybir.AluOpType
PM = mybir.MatmulPerfMode

W_SCALE = 64.0   # host pre-scale on W1 before e4m3 quantization
RES_KK = 8       # kk chunks (of KK) that get the W-residual pass (8 = all)
SPE = 12         # last-batch pass2: PE covers tokens [0, SPE*128)


def build_program(bpc=BPC, s=S, d=D, h=H, nt=512, nhalf=2, res_kk=RES_KK,
                  spe=SPE):
    """Build the per-core Bass program (SPMD; identical on all cores)."""
    P = 128
    KK = d // 256          # DoubleRow chunks (256-deep contraction each)
    KD = d // P            # fp16 pass2 d-chunks
    MH = h // P            # h chunks
    sh = s // nhalf        # tokens per s-half (stream tile granularity)
    assert sh % nt == 0 and d % 256 == 0 and h % P == 0
    NTH = sh // nt         # token tiles per half
    assert spe * P >= sh, "DVE share of the last batch must fit in half 1"
    dve_off = spe * P - sh      # token offset of DVE share within half 1
    dve_w = s - spe * P         # DVE share width (tokens)

    nc = bacc.Bacc(None, target_bir_lowering=False)
    enc8 = nc.declare_dram_parameter("enc8", [bpc, d, s], FP8, isOutput=False)
    enc16 = nc.declare_dram_parameter("enc16", [bpc, d, s], F16,
                                      isOutput=False)
    # natural-layout fp16 rows [0, spe*P) of the core's LAST batch, for the
    # PE share of its pass2
    encn = nc.declare_dram_parameter("encn", [spe * P, d], F16,
                                     isOutput=False)
    w8a = nc.declare_dram_parameter("w8a", [d, h], FP8, isOutput=False)
    w8b = nc.declare_dram_parameter("w8b", [d, h], FP8, isOutput=False)
    vt = nc.declare_dram_parameter("vt", [h], F16, isOutput=False)
    cbias = nc.declare_dram_parameter("cbias", [h, bpc], F32, isOutput=False)
    pen = nc.declare_dram_parameter("pen", [bpc, s], BF16, isOutput=False)
    ctx_out = nc.declare_dram_parameter("ctx", [bpc, d], F32, isOutput=True)
    # PE share of the last batch's context; host adds it into ctx[bpc-1]
    ctxpe_out = nc.declare_dram_parameter("ctxpe", [d], F32, isOutput=True)
    attn_dram = nc.dram_tensor("attn_bounce", [s], F32)

    with tile.TileContext(nc) as tc:
        with (
            tc.tile_pool(name="singles", bufs=1) as singles,
            tc.tile_pool(name="et8_pool", bufs=3) as et8_pool,
            tc.tile_pool(name="et16_pool", bufs=2) as et16_pool,
            tc.tile_pool(name="en_pool", bufs=3) as en_pool,
            tc.tile_pool(name="row_pool", bufs=1) as row_pool,
            tc.tile_pool(name="pen_pool", bufs=2) as pen_pool,
            tc.tile_pool(name="bc_pool", bufs=2) as bc_pool,
            tc.tile_pool(name="scr_pool", bufs=2) as scr_pool,
            tc.tile_pool(name="ctx_pool", bufs=2) as ctx_pool,
            tc.tile_pool(name="stat_pool", bufs=4) as stat_pool,
            tc.tile_pool(name="psum_mm", bufs=2, space="PSUM") as psum_mm,
            tc.tile_pool(name="psum_sc", bufs=2, space="PSUM") as psum_sc,
            tc.tile_pool(name="psum_ctx", bufs=1, space="PSUM") as psum_ctx,
        ):
            # Resident constants on the Pool/SWDGE queue so the SP queue can
            # start streaming enc immediately.
            w8a_sb = singles.tile([P, KK, 2, h], FP8)
            w8b_sb = singles.tile([P, KK, 2, h], FP8)
            w8a_r = w8a.rearrange("(kk i p) h -> p kk i h", p=P, i=2)
            w8b_r = w8b.rearrange("(kk i p) h -> p kk i h", p=P, i=2)
            for k in range(0, KK, 2):
                nc.gpsimd.dma_start(w8a_sb[:, k:k + 2], w8a_r[:, k:k + 2])
            for k in range(0, KK, 2):
                nc.gpsimd.dma_start(w8b_sb[:, k:k + 2], w8b_r[:, k:k + 2])
            vt_sb = singles.tile([P, MH], F16)
            nc.gpsimd.dma_start(vt_sb, vt.rearrange("(m p) -> p m", p=P))
            cb_sb = singles.tile([P, MH, bpc], F32)
            nc.gpsimd.dma_start(cb_sb, cbias.rearrange("(m p) b -> p m b", p=P))

            for b in range(bpc):
                pen_row = pen_pool.tile([1, s], BF16, tag="pen")
                nc.gpsimd.dma_start(pen_row, pen[b][None, :])

                row = row_pool.tile([1, s], F32, tag="row")
                ets16 = []
                for hf in range(nhalf):
                    # fp8 transposed tiles (pass1):
                    # et8[p, kk, i, t] = enc8[b, kk*256 + i*128 + p, hf*sh+t]
                    et8 = et8_pool.tile([P, KK, 2, sh], FP8, tag="et8")
                    for kc in range(0, KK, 2):
                        nc.sync.dma_start(
                            et8[:, kc:kc + 2],
                            enc8[
                                b, kc * 256:(kc + 2) * 256,
                                hf * sh:(hf + 1) * sh,
                            ].rearrange("(kk i p) t -> p kk i t", p=P, i=2),
                        )
                    # fp16 transposed tiles (pass2) on the ACT HWDGE queue
                    et16 = et16_pool.tile([P, KD, sh], F16, tag="et16")
                    for kc in range(0, KD, 8):
                        nc.scalar.dma_start(
                            et16[:, kc:kc + 8],
                            enc16[
                                b, kc * P:(kc + 8) * P,
                                hf * sh:(hf + 1) * sh,
                            ].rearrange("(k p) t -> p k t", p=P),
                        )
                    ets16.append(et16)

                    for n in range(NTH):
                        ng = hf * NTH + n  # global token-tile index
                        nsl = slice(n * nt, (n + 1) * nt)
                        ps_sc = psum_sc.tile([1, nt], F32)
                        pending = None  # (m, energy) awaiting scores matmul
                        for m in range(MH):
                            ps = psum_mm.tile([P, nt], F32)
                            msl = slice(m * P, (m + 1) * P)
                            for kk in range(KK):
                                nc.tensor.matmul(
                                    ps,
                                    w8a_sb[:, kk, :, msl],
                                    et8[:, kk, :, nsl],
                                    start=(kk == 0),
                                    stop=(kk == KK - 1 and res_kk == 0),
                                    perf_mode=PM.DoubleRow,
                                )
                            for kk in range(res_kk):
                                nc.tensor.matmul(
                                    ps,
                                    w8b_sb[:, kk, :, msl],
                                    et8[:, kk, :, nsl],
                                    start=False,
                                    stop=(kk == res_kk - 1),
                                    perf_mode=PM.DoubleRow,
                                )
                            # scores for the PREVIOUS m: issued after this
                            # m's pass1 group so the in-order PE queue never
                            # waits on ACT's tanh.
                            if pending is not None:
                                pm_, pen_energy = pending
                                nc.tensor.matmul(
                                    ps_sc,
                                    vt_sb[:, pm_:pm_ + 1],
                                    pen_energy,
                                    start=(pm_ == 0),
                                    stop=False,
                                )
                            energy = en_pool.tile([P, nt], F16, tag="energy")
                            nc.scalar.activation(
                                energy, ps, AF.Tanh,
                                bias=cb_sb[:, m, b:b + 1],
                                scale=1.0 / W_SCALE,
                            )
                            pending = (m, energy)
                        nc.tensor.matmul(
                            ps_sc,
                            vt_sb[:, MH - 1:MH],
                            pending[1],
                            start=False,
                            stop=True,
                        )
                        # scores(+V_b, +mask penalty) into the batch row
                        nc.vector.tensor_tensor(
                            row[:, ng * nt:(ng + 1) * nt],
                            ps_sc,
                            pen_row[:, ng * nt:(ng + 1) * nt],
                            ALU.add,
                        )

                # Softmax over the full row (in place: row -> exp -> attn).
                negmax = stat_pool.tile([1, 1], F32, tag="negmax")
                nc.vector.tensor_reduce(
                    negmax, row, axis=mybir.AxisListType.X, op=ALU.max,
                    negate=True,
                )
                ssum = stat_pool.tile([1, 1], F32, tag="ssum")
                nc.scalar.activation(
                    row, row, AF.Exp, bias=negmax, scale=1.0, accum_out=ssum,
                )
                rinv = stat_pool.tile([1, 1], F32, tag="rinv")
                nc.vector.reciprocal(rinv, ssum)
                nc.vector.tensor_scalar_mul(row, row, rinv)

                # attn bounce to DRAM on the ACT HWDGE queue: it sits between
                # et16[b] and et16[b+1] in FIFO order, and et16[b+1] isn't
                # needed until well after softmax completes, so no stall.
                nc.scalar.dma_start(attn_dram[None, :], row)

                if b < bpc - 1:
                    # Broadcast attn across partitions via a replicated
                    # (partition-step-0) SWDGE read, cast f32 -> f16.
                    attn_bc = bc_pool.tile([P, s], F16, tag="attn_bc")
                    attn_src = attn_dram[None, :]
                    attn_src = bass.AP(
                        tensor=attn_src.tensor,
                        offset=attn_src.offset,
                        ap=[[0, P]] + list(attn_src.ap[1:]),
                    )
                    nc.gpsimd.dma_start(attn_bc, attn_src)

                    # Pass 2: fused multiply+accumulate on the DVE over the
                    # resident fp16 transposed tiles, hidden under the next
                    # batch's pass1.
                    ctx_sb = ctx_pool.tile([P, KD], F32, tag="ctx")
                    for hf in range(nhalf):
                        hsl = slice(hf * sh, (hf + 1) * sh)
                        for k in range(KD):
                            scratch = scr_pool.tile(
                                [P, sh], F16, tag="scratch"
                            )
                            part = stat_pool.tile([P, 1], F32, tag="part")
                            nc.vector.scalar_tensor_tensor(
                                scratch, ets16[hf][:, k, :], 1.0,
                                attn_bc[:, hsl], ALU.mult, ALU.mult,
                                accum_out=part,
                            )
                            if hf == 0:
                                nc.vector.tensor_copy(
                                    ctx_sb[:, k:k + 1], part
                                )
                            else:
                                nc.vector.tensor_tensor(
                                    ctx_sb[:, k:k + 1], ctx_sb[:, k:k + 1],
                                    part, ALU.add,
                                )
                    nc.gpsimd.dma_start(
                        ctx_out[b].rearrange("(k p) -> p k", p=P), ctx_sb,
                    )
                else:
                    # Last batch: split pass2 between the now-idle PE
                    # (tokens [0, spe*P), natural-layout slice) and the DVE
                    # (remaining tokens); host sums the two partials.
                    NJ = 3
                    skg = spe // NJ
                    ents = []
                    for j in range(NJ):
                        ent = et8_pool.tile([P, skg, d], F16, tag="et8")
                        nc.sync.dma_start(
                            ent,
                            encn[j * skg * P:(j + 1) * skg * P, :].rearrange(
                                "(c p) dd -> p c dd", p=P
                            ),
                        )
                        ents.append(ent)

                    # attn for the PE share, partition-major
                    attn_part = stat_pool.tile([P, spe], F16, tag="attn_part")
                    nc.gpsimd.dma_start(
                        attn_part,
                        attn_dram[:spe * P].rearrange("(sk p) -> p sk", p=P),
                    )
                    # attn for the DVE share, broadcast across partitions
                    attn_bc = bc_pool.tile([P, dve_w], F16, tag="attn_bc2")
                    attn_src = attn_dram[None, spe * P:]
                    attn_src = bass.AP(
                        tensor=attn_src.tensor,
                        offset=attn_src.offset,
                        ap=[[0, P]] + list(attn_src.ap[1:]),
                    )
                    nc.gpsimd.dma_start(attn_bc, attn_src)

                    ctx_ps = psum_ctx.tile([1, d], F32)
                    for j in range(NJ):
                        for c in range(skg):
                            sk = j * skg + c
                            for dt_ in range(d // nt):
                                nc.tensor.matmul(
                                    ctx_ps[:, dt_ * nt:(dt_ + 1) * nt],
                                    attn_part[:, sk:sk + 1],
                                    ents[j][:, c, dt_ * nt:(dt_ + 1) * nt],
                                    start=(sk == 0),
                                    stop=(sk == spe - 1),
                                    skip_group_check=(nt * 4 >= 2048),
                                )
                    ctx_row = ctx_pool.tile([1, d], F32, tag="ctxrow")
                    nc.scalar.activation(ctx_row, ctx_ps, AF.Copy, scale=1.0)
                    nc.scalar.dma_start(ctxpe_out[None, :], ctx_row)

                    ctx_sb = ctx_pool.tile([P, KD], F32, tag="ctx")
                    for k in range(KD):
                        scratch = scr_pool.tile([P, sh], F16, tag="scratch")
                        part = stat_pool.tile([P, 1], F32, tag="part")
                        nc.vector.scalar_tensor_tensor(
                            scratch[:, :dve_w],
                            ets16[1][:, k, dve_off:dve_off + dve_w], 1.0,
                            attn_bc, ALU.mult, ALU.mult,
                            accum_out=part,
                        )
                        nc.vector.tensor_copy(ctx_sb[:, k:k + 1], part)
                    nc.gpsimd.dma_start(
                        ctx_out[b].rearrange("(k p) -> p k", p=P), ctx_sb,
                    )
    nc.finalize()
    return nc


_PROGRAM_CACHE = {}


def _get_program(key, **kwargs):
    if key not in _PROGRAM_CACHE:
        _PROGRAM_CACHE[key] = build_program(**kwargs)
    return _PROGRAM_CACHE[key]


def prep_inputs(enc_output, enc_mask, dec_hidden, W_w, W_b, V_w, V_b):
    """Host-side shard + prep: returns per-core in_maps."""
    enc = np.asarray(enc_output, dtype=np.float32)
    mask = np.asarray(enc_mask, dtype=np.float32)[..., 0]          # (B, S)
    dec = np.asarray(dec_hidden, dtype=np.float32)[0]              # (B, H)
    W = np.asarray(W_w, dtype=np.float32)                          # (H, 3H)
    Wb = np.asarray(W_b, dtype=np.float32)                         # (H,)
    V = np.asarray(V_w, dtype=np.float32)[0]                       # (H,)
    Vb = float(np.asarray(V_b, dtype=np.float32)[0])

    enc_t = np.ascontiguousarray(enc.transpose(0, 2, 1))           # (B, D, S)
    enc8 = enc_t.astype(ml_dtypes.float8_e4m3)
    enc16 = enc_t.astype(np.float16)

    w1t = np.ascontiguousarray(W[:, :D].T) * W_SCALE               # (D, H)
    w8a = w1t.astype(ml_dtypes.float8_e4m3)
    w8b = (w1t - w8a.astype(np.float32)).astype(ml_dtypes.float8_e4m3)

    # Tiny dec projection folded into a per-(h, b) bias (0.01% of FLOPs).
    cbias_all = (dec @ W[:, D:].T + Wb).astype(np.float32)         # (B, H)
    pen_all = (np.where(mask > 0, 0.0, -1e30) + Vb).astype(
        ml_dtypes.bfloat16)                                        # (B, S)

    in_maps = []
    for c in range(NCORES):
        sl = slice(c * BPC, (c + 1) * BPC)
        in_maps.append({
            "enc8": enc8[sl],
            "enc16": enc16[sl],
            "encn": np.ascontiguousarray(
                enc[c * BPC + BPC - 1, :SPE * 128, :]).astype(np.float16),
            "w8a": w8a,
            "w8b": w8b,
            "vt": V.astype(np.float16),
            "cbias": np.ascontiguousarray(cbias_all[sl].T),        # (H, BPC)
            "pen": np.ascontiguousarray(pen_all[sl]),
        })
    return in_maps


def kernel(**inputs) -> np.ndarray:
    in_maps = prep_inputs(**inputs)
    nc = _get_program("full")
    res = run_bass_kernel_spmd(nc, in_maps, list(range(NCORES)))
    outs = []
    for c in range(NCORES):
        ctx = res.results[c]["ctx"].astype(np.float32).copy()
        ctx[BPC - 1] += res.results[c]["ctxpe"].astype(np.float32)
        outs.append(ctx)
    return np.ascontiguousarray(np.concatenate(outs, axis=0))


if __name__ == "__main__":
    rng = np.random.default_rng(0)
    inputs = {
        "enc_output": rng.standard_normal((B, S, D), dtype=np.float32),
        "enc_mask": np.ones((B, S, 1), dtype=np.float32),
        "dec_hidden": rng.standard_normal((1, B, H), dtype=np.float32),
        "W_w": (rng.standard_normal((H, 3 * H), dtype=np.float32)
                / np.sqrt(3 * H)),
        "W_b": np.zeros((H,), dtype=np.float32),
        "V_w": rng.standard_normal((1, H), dtype=np.float32) / np.sqrt(H),
        "V_b": np.zeros((1,), dtype=np.float32),
    }
    out = kernel(**inputs)
    print(out.shape, out.dtype, float(np.abs(out).mean()))
